# revision 3
# baseline (speedup 1.0000x reference)
"""Causal self-attention with RoPE on 8 trn2 NeuronCores.

Sharding: heads 2r,2r+1 -> core r (both batches). w_attn column-sharded
(rows permuted even/odd per head so interleaved RoPE becomes rotate-half);
attention computed per-core in transposed [tk, tq] score layout; AllToAll
re-shards heads->sequence so each core runs the full output projection for
its own 512-token slice. Host only slices/permutes inputs and concatenates
the 8 output slices.
"""

import math
import os
import sys
import tempfile

if "/opt/trn_rl_repo" not in sys.path:
    sys.path.insert(0, "/opt/trn_rl_repo")

import numpy as np

import concourse.bacc as bacc
import concourse.bass as bass
import concourse.mybir as mybir
import concourse.tile as tile
from concourse.bass_utils import run_bass_kernel_spmd

B, T, D = 2, 2048, 2048
H, HD = 16, 128
NCORES = 8
HL = H // NCORES          # heads per core
BT = B * T
TS = 512                  # t supertile (psum bank width in fp32)
NTB = T // TS             # supertiles per batch
NE = D // 128             # e-chunks (contraction) in qkv
NKC = T // 128            # tk chunks per batch
FQKV = 3 * HL * HD        # 768 qkv features per core
FP = mybir.dt.float32
FPR = mybir.dt.float32r
SCALE = 1.0 / math.sqrt(HD)

LAST_EXEC_NS = None
LAST_TRACE = None

_built = {}


def _install_ntff_shim():
    import types

    import antenv

    if "antenv.axon_hooks" not in sys.modules:
        mod = types.ModuleType("antenv.axon_hooks")
        _hook = [None]
        mod.set_axon_ntff_profile_hook = lambda h: _hook.__setitem__(0, h)
        mod.get_axon_ntff_profile_hook = lambda: _hook[0]
        sys.modules["antenv.axon_hooks"] = mod
        antenv.axon_hooks = mod
    from antenv.axon_hooks import (
        get_axon_ntff_profile_hook,
        set_axon_ntff_profile_hook,
    )

    if get_axon_ntff_profile_hook() is None:
        from trn_agent_boot.trn_boot import _ntff_profile_via_ctypes

        set_axon_ntff_profile_hook(_ntff_profile_via_ctypes("/opt/axon/libaxon_pjrt.so"))
    import concourse.bass_utils as bu

    bu.upload_artifacts = lambda tmpdir: f"local:{tmpdir}"


def _build():
    if "nc" in _built:
        return _built["nc"]
    nc = bacc.Bacc("TRN2", target_bir_lowering=False, debug=False, num_devices=NCORES)

    xT = nc.dram_tensor("xT", [D, BT], FPR, kind="ExternalInput")
    wT = nc.dram_tensor("wT", [D, FQKV], FPR, kind="ExternalInput")
    wpT = nc.dram_tensor("wpT", [D, D], FPR, kind="ExternalInput")
    cs2 = nc.dram_tensor("cs2", [128, T], FP, kind="ExternalInput")
    sn2 = nc.dram_tensor("sn2", [128, T], FP, kind="ExternalInput")
    out_loc = nc.dram_tensor("out_loc", [TS, D], FP, kind="ExternalOutput")

    from contextlib import ExitStack

    with tile.TileContext(nc) as tc:
        with ExitStack() as whole:
            dpool = whole.enter_context(tc.tile_pool(name="dram", bufs=1, space="DRAM"))
            a2a_in = dpool.tile([NCORES, HL * HD, TS], FPR, name="a2a_in")
            a2a_out = dpool.tile([NCORES, HL * HD, TS], FPR, name="a2a_out")
            psum = whole.enter_context(tc.tile_pool(name="psum", bufs=1, space="PSUM"))
            smallp = whole.enter_context(tc.tile_pool(name="small", bufs=1))
            ones_f = smallp.tile([128, 1], FP, name="ones_f")
            nc.vector.memset(ones_f[:], 1.0)
            ones = smallp.tile([128, 1], FPR, name="ones")
            nc.vector.tensor_copy(out=ones[:], in_=ones_f[:])

            with ExitStack() as ph12:
                tabs = ph12.enter_context(tc.tile_pool(name="tabs", bufs=1))
                cs_sb = tabs.tile([128, T], FP, name="cs_sb")
                sn_sb = tabs.tile([128, T], FP, name="sn_sb")
                nc.sync.dma_start(out=cs_sb[:], in_=cs2[:])
                nc.sync.dma_start(out=sn_sb[:], in_=sn2[:])

                wq = ph12.enter_context(tc.tile_pool(name="wq", bufs=1))
                w_sb = wq.tile([128, NE, FQKV], FPR, name="w_sb")
                nc.sync.dma_start(
                    out=w_sb[:], in_=wT[:].rearrange("(c p) f -> p c f", p=128)
                )

                store = ph12.enter_context(tc.tile_pool(name="store", bufs=1))
                xtp = ph12.enter_context(tc.tile_pool(name="xt", bufs=4))
                work = ph12.enter_context(tc.tile_pool(name="work", bufs=2))
                exps = ph12.enter_context(tc.tile_pool(name="exps", bufs=3))
                osbp = ph12.enter_context(tc.tile_pool(name="osb", bufs=2))
                bcp = ph12.enter_context(tc.tile_pool(name="bc", bufs=2))
                recp = ph12.enter_context(tc.tile_pool(name="rec", bufs=2))

                for b in range(B):
                    qrot = [
                        store.tile([128, T], FPR, tag=f"qrot{hl}", name=f"qrot{hl}_{b}")
                        for hl in range(HL)
                    ]
                    krot = [
                        store.tile([128, T], FPR, tag=f"krot{hl}", name=f"krot{hl}_{b}")
                        for hl in range(HL)
                    ]
                    v_all = store.tile(
                        [128, HL, NKC, HD], FPR, tag="v_all", name=f"v_all_{b}"
                    )

                    # ---- phase 1: qkv projection + rope (per tb supertile) ----
                    for tb in range(NTB):
                        toff = b * T + tb * TS
                        pqk = [
                            psum.tile([128, TS], FP, tag=f"bank{g}", name=f"qk{g}_{b}_{tb}")
                            for g in range(4)
                        ]
                        pv = [
                            psum.tile(
                                [128, 2 * HD], FP, tag=f"bank{4 + st}", name=f"v{st}_{b}_{tb}"
                            )
                            for st in range(4)
                        ]
                        for e in range(NE):
                            xt_t = xtp.tile([128, TS], FPR, tag="xt", name=f"xt_{b}_{tb}_{e}")
                            nc.sync.dma_start(
                                out=xt_t[:],
                                in_=xT[e * 128 : (e + 1) * 128, toff : toff + TS],
                            )
                            for g in range(4):
                                nc.tensor.matmul(
                                    pqk[g][:],
                                    lhsT=w_sb[:, e, g * 128 : (g + 1) * 128],
                                    rhs=xt_t[:],
                                    start=(e == 0),
                                    stop=(e == NE - 1),
                                    skip_group_check=True,
                                )
                            for st in range(4):
                                nc.tensor.matmul(
                                    pv[st][:],
                                    lhsT=xt_t[:, st * 128 : (st + 1) * 128],
                                    rhs=w_sb[:, e, 512:768],
                                    start=(e == 0),
                                    stop=(e == NE - 1),
                                    skip_group_check=True,
                                )
                        # rope: (even, odd) psum pairs -> rotated, assembled per head
                        csl = cs_sb[:, tb * TS : (tb + 1) * TS]
                        snl = sn_sb[:, tb * TS : (tb + 1) * TS]
                        for dst, pe_, po_ in ((qrot, pqk[0], pqk[1]), (krot, pqk[2], pqk[3])):
                            a_ = work.tile([128, TS], FP, tag="w0", name=f"a_{b}_{tb}")
                            b_ = work.tile([128, TS], FP, tag="w1", name=f"b_{b}_{tb}")
                            c_ = work.tile([128, TS], FP, tag="w2", name=f"c_{b}_{tb}")
                            d_ = work.tile([128, TS], FP, tag="w3", name=f"d_{b}_{tb}")
                            nc.vector.tensor_tensor(a_[:], pe_[:], csl, mybir.AluOpType.mult)
                            nc.vector.tensor_tensor(b_[:], po_[:], snl, mybir.AluOpType.mult)
                            nc.vector.tensor_tensor(c_[:], pe_[:], snl, mybir.AluOpType.mult)
                            nc.vector.tensor_tensor(d_[:], po_[:], csl, mybir.AluOpType.mult)
                            tsl = slice(tb * TS, (tb + 1) * TS)
                            for hl in range(HL):
                                hs = slice(hl * 64, (hl + 1) * 64)
                                nc.vector.tensor_tensor(
                                    dst[hl][0:64, tsl], a_[hs, :], b_[hs, :],
                                    mybir.AluOpType.subtract,
                                )
                                nc.vector.tensor_tensor(
                                    dst[hl][64:128, tsl], c_[hs, :], d_[hs, :],
                                    mybir.AluOpType.add,
                                )
                        for st in range(4):
                            j = tb * 4 + st
                            for hl in range(HL):
                                nc.vector.tensor_copy(
                                    out=v_all[:, hl, j, :],
                                    in_=pv[st][:, hl * HD : (hl + 1) * HD],
                                )

                    # ---- phase 2: attention for this batch ----
                    for hl in range(HL):
                        for tb in range(NTB):
                            po_ = psum.tile([128, TS], FP, tag="bank2", name=f"o_{b}_{hl}_{tb}")
                            psum_s = psum.tile([1, TS], FP, tag="bank3", name=f"sm_{b}_{hl}_{tb}")
                            nj = 4 * tb + 4
                            for j in range(nj):
                                sp = psum.tile(
                                    [128, TS], FP, tag=f"bank{j % 2}", name=f"s_{b}_{hl}_{tb}_{j}"
                                )
                                nc.tensor.matmul(
                                    sp[:],
                                    lhsT=krot[hl][:, j * 128 : (j + 1) * 128],
                                    rhs=qrot[hl][:, tb * TS : (tb + 1) * TS],
                                    start=True,
                                    stop=True,
                                    skip_group_check=True,
                                )
                                ex = exps.tile([128, TS], FPR, tag="exp", name=f"e_{b}_{hl}_{tb}_{j}")
                                nc.scalar.activation(
                                    out=ex[:], in_=sp[:],
                                    func=mybir.ActivationFunctionType.Exp, scale=SCALE,
                                )
                                if j >= 4 * tb:
                                    nc.gpsimd.affine_select(
                                        out=ex[:], in_=ex[:],
                                        pattern=[[1, TS]],
                                        compare_op=mybir.AluOpType.is_ge,
                                        fill=0.0,
                                        base=tb * TS - j * 128,
                                        channel_multiplier=-1,
                                    )
                                nc.tensor.matmul(
                                    po_[:],
                                    lhsT=v_all[:, hl, j, :],
                                    rhs=ex[:],
                                    start=(j == 0),
                                    stop=(j == nj - 1),
                                    skip_group_check=True,
                                )
                                nc.tensor.matmul(
                                    psum_s[:],
                                    lhsT=ones[:],
                                    rhs=ex[:],
                                    start=(j == 0),
                                    stop=(j == nj - 1),
                                    skip_group_check=True,
                                )
                            rec = recp.tile([1, TS], FP, tag="rec", name=f"r_{b}_{hl}_{tb}")
                            nc.vector.reciprocal(out=rec[:], in_=psum_s[:])
                            bc = bcp.tile([128, TS], FP, tag="bc", name=f"bc_{b}_{hl}_{tb}")
                            nc.gpsimd.partition_broadcast(bc[:], rec[:])
                            osb = osbp.tile([128, TS], FPR, tag="osb", name=f"ot_{b}_{hl}_{tb}")
                            nc.vector.tensor_tensor(osb[:], po_[:], bc[:], mybir.AluOpType.mult)
                            nc.sync.dma_start(
                                out=a2a_in[b * NTB + tb, hl * HD : (hl + 1) * HD, :],
                                in_=osb[:],
                            )

            # ---- all-to-all: heads -> sequence re-shard ----
            nc.gpsimd.collective_compute(
                "AllToAll",
                mybir.AluOpType.bypass,
                replica_groups=[list(range(NCORES))],
                ins=[a2a_in[:].opt()],
                outs=[a2a_out[:].opt()],
            )

            # ---- phase 3: output projection for the local 512-token slice ----
            with ExitStack() as ph3:
                yp = ph3.enter_context(tc.tile_pool(name="yp", bufs=1))
                wpp = ph3.enter_context(tc.tile_pool(name="wpp", bufs=3))
                outp = ph3.enter_context(tc.tile_pool(name="outp", bufs=2))
                y = []
                for ci in range(NE):
                    yt = yp.tile([128, TS], FPR, tag=f"y{ci}", name=f"y{ci}")
                    nc.sync.dma_start(
                        out=yt[:],
                        in_=a2a_out[ci // HL, (ci % HL) * HD : (ci % HL + 1) * HD, :],
                    )
                    y.append(yt)
                for dq in range(4):
                    pp = [
                        psum.tile([128, TS], FP, tag=f"bank{t2}", name=f"pp{t2}_{dq}")
                        for t2 in range(4)
                    ]
                    for ci in range(NE):
                        wpt = wpp.tile([128, TS], FPR, tag="wp", name=f"wp_{dq}_{ci}")
                        nc.sync.dma_start(
                            out=wpt[:],
                            in_=wpT[ci * 128 : (ci + 1) * 128, dq * TS : (dq + 1) * TS],
                        )
                        for t2 in range(4):
                            nc.tensor.matmul(
                                pp[t2][:],
                                lhsT=y[ci][:, t2 * 128 : (t2 + 1) * 128],
                                rhs=wpt[:],
                                start=(ci == 0),
                                stop=(ci == NE - 1),
                                skip_group_check=True,
                            )
                    for t2 in range(4):
                        ob = outp.tile([128, TS], FP, tag="ob", name=f"ob_{dq}_{t2}")
                        nc.vector.tensor_copy(out=ob[:], in_=pp[t2][:])
                        nc.sync.dma_start(
                            out=out_loc[t2 * 128 : (t2 + 1) * 128, dq * TS : (dq + 1) * TS],
                            in_=ob[:],
                        )

    nc.compile()
    _built["nc"] = nc
    return nc


def _host_prep(x, w_attn, w_proj):
    x2 = np.ascontiguousarray(x.reshape(BT, D).T)  # [D, BT] e-major
    wpT_full = np.ascontiguousarray(w_proj.T)      # [c, d]

    inv = 1.0 / (10000.0 ** (np.arange(0, HD, 2, dtype=np.float32) / HD))
    t = np.arange(T, dtype=np.float32)
    fr = np.outer(t, inv)                          # [T, 64]
    cosT = np.cos(fr).T.astype(np.float32)         # [64, T]
    sinT = np.sin(fr).T.astype(np.float32)
    cs2v = np.ascontiguousarray(np.vstack([cosT, cosT]))
    sn2v = np.ascontiguousarray(np.vstack([sinT, sinT]))

    perm = np.concatenate([np.arange(0, HD, 2), np.arange(1, HD, 2)])
    in_maps = []
    for r in range(NCORES):
        h0, h1 = HL * r, HL * r + 1
        rows = []
        for off in (0, D):  # q block then k block
            rows += [off + h0 * HD + perm[:64], off + h1 * HD + perm[:64]]
            rows += [off + h0 * HD + perm[64:], off + h1 * HD + perm[64:]]
        rows += [2 * D + h0 * HD + np.arange(HD), 2 * D + h1 * HD + np.arange(HD)]
        w_c = w_attn[np.concatenate(rows)]         # [768, D]
        wT_c = np.ascontiguousarray(w_c.T)         # [D, 768]
        in_maps.append(
            {"xT": x2, "wT": wT_c, "wpT": wpT_full, "cs2": cs2v, "sn2": sn2v}
        )
    return in_maps


def kernel(x, w_attn, w_proj):
    global LAST_EXEC_NS, LAST_TRACE
    x = np.asarray(x, dtype=np.float32)
    w_attn = np.asarray(w_attn, dtype=np.float32)
    w_proj = np.asarray(w_proj, dtype=np.float32)

    trace = os.environ.get("KERNEL_TRACE") == "1"
    if trace:
        _install_ntff_shim()

    nc = _build()
    in_maps = _host_prep(x, w_attn, w_proj)
    kw = {}
    if trace:
        tmpdir = os.environ.get("KERNEL_TRACE_DIR") or tempfile.mkdtemp(prefix="ktrace_")
        kw = dict(trace=True, tmpdir=tmpdir)
        LAST_TRACE = tmpdir
    res = run_bass_kernel_spmd(nc, in_maps, list(range(NCORES)), **kw)
    LAST_EXEC_NS = res.exec_time_ns

    out = np.empty((B, T, D), dtype=np.float32)
    for r in range(NCORES):
        b, tb = divmod(r, NTB)
        out[b, tb * TS : (tb + 1) * TS, :] = res.results[r]["out_loc"]
    return out


# revision 4
# speedup vs baseline: 1.4091x; 1.4091x over previous
"""Causal self-attention with RoPE on 8 trn2 NeuronCores.

Sharding: heads 2r,2r+1 -> core r (both batches). w_attn column-sharded
(rows permuted even/odd per head so interleaved RoPE becomes rotate-half);
attention computed per-core in transposed [tk, tq] score layout; AllToAll
re-shards heads->sequence so each core runs the full output projection for
its own 512-token slice. Host only slices/permutes/casts inputs and
concatenates the 8 output slices. Matmul operands are bf16 (fp32 PSUM
accumulation); fp32 everywhere else.
"""

import math
import os
import sys
import tempfile

if "/opt/trn_rl_repo" not in sys.path:
    sys.path.insert(0, "/opt/trn_rl_repo")

import ml_dtypes
import numpy as np

import concourse.bacc as bacc
import concourse.bass as bass
import concourse.mybir as mybir
import concourse.tile as tile
from concourse.bass_utils import run_bass_kernel_spmd

B, T, D = 2, 2048, 2048
H, HD = 16, 128
NCORES = 8
HL = H // NCORES          # heads per core
BT = B * T
TS = 512                  # t supertile (psum bank width in fp32)
NTB = T // TS             # supertiles per batch
NE = D // 128             # e-chunks (contraction) in qkv
NKC = T // 128            # tk chunks per batch
FQKV = 3 * HL * HD        # 768 qkv features per core
FP = mybir.dt.float32
BF = mybir.dt.bfloat16
SCALE = 1.0 / math.sqrt(HD)

LAST_EXEC_NS = None
LAST_TRACE = None

_built = {}


def _install_ntff_shim():
    import types

    import antenv

    if "antenv.axon_hooks" not in sys.modules:
        mod = types.ModuleType("antenv.axon_hooks")
        _hook = [None]
        mod.set_axon_ntff_profile_hook = lambda h: _hook.__setitem__(0, h)
        mod.get_axon_ntff_profile_hook = lambda: _hook[0]
        sys.modules["antenv.axon_hooks"] = mod
        antenv.axon_hooks = mod
    from antenv.axon_hooks import (
        get_axon_ntff_profile_hook,
        set_axon_ntff_profile_hook,
    )

    if get_axon_ntff_profile_hook() is None:
        from trn_agent_boot.trn_boot import _ntff_profile_via_ctypes

        set_axon_ntff_profile_hook(_ntff_profile_via_ctypes("/opt/axon/libaxon_pjrt.so"))
    import concourse.bass_utils as bu

    bu.upload_artifacts = lambda tmpdir: f"local:{tmpdir}"


def _build():
    if "nc" in _built:
        return _built["nc"]
    nc = bacc.Bacc("TRN2", target_bir_lowering=False, debug=False, num_devices=NCORES)

    xT = nc.dram_tensor("xT", [D, BT], BF, kind="ExternalInput")
    wT = nc.dram_tensor("wT", [D, FQKV], BF, kind="ExternalInput")
    wpT = nc.dram_tensor("wpT", [D, D], BF, kind="ExternalInput")
    cs2 = nc.dram_tensor("cs2", [128, T], FP, kind="ExternalInput")
    sn2 = nc.dram_tensor("sn2", [128, T], FP, kind="ExternalInput")
    out_loc = nc.dram_tensor("out_loc", [TS, D], FP, kind="ExternalOutput")

    from contextlib import ExitStack

    with tile.TileContext(nc) as tc:
        with ExitStack() as whole:
            dpool = whole.enter_context(tc.tile_pool(name="dram", bufs=1, space="DRAM"))
            a2a_in = dpool.tile([NCORES, HL * HD, TS], BF, name="a2a_in")
            a2a_out = dpool.tile([NCORES, HL * HD, TS], BF, name="a2a_out")
            psum = whole.enter_context(tc.tile_pool(name="psum", bufs=1, space="PSUM"))
            smallp = whole.enter_context(tc.tile_pool(name="small", bufs=1))
            ones_f = smallp.tile([128, 1], FP, name="ones_f")
            nc.vector.memset(ones_f[:], 1.0)
            ones = smallp.tile([128, 1], BF, name="ones")
            nc.vector.tensor_copy(out=ones[:], in_=ones_f[:])

            wq = whole.enter_context(tc.tile_pool(name="wq", bufs=1))
            w_sb = wq.tile([128, NE, FQKV], BF, name="w_sb")
            for c4 in range(4):
                nc.sync.dma_start(
                    out=w_sb[:, c4 * 4 : (c4 + 1) * 4, :],
                    in_=wT[c4 * 512 : (c4 + 1) * 512, :].rearrange(
                        "(c p) f -> p c f", p=128
                    ),
                )

            tabs = whole.enter_context(tc.tile_pool(name="tabs", bufs=1))
            cs_sb = tabs.tile([128, T], FP, name="cs_sb")
            sn_sb = tabs.tile([128, T], FP, name="sn_sb")
            nc.sync.dma_start(out=cs_sb[:], in_=cs2[:])
            nc.sync.dma_start(out=sn_sb[:], in_=sn2[:])

            store = whole.enter_context(tc.tile_pool(name="store", bufs=1))
            xtp = whole.enter_context(tc.tile_pool(name="xt", bufs=2))
            work = whole.enter_context(tc.tile_pool(name="work", bufs=2))
            exps = whole.enter_context(tc.tile_pool(name="exps", bufs=3))
            osbp = whole.enter_context(tc.tile_pool(name="osb", bufs=2))
            bcp = whole.enter_context(tc.tile_pool(name="bc", bufs=2))
            recp = whole.enter_context(tc.tile_pool(name="rec", bufs=2))
            yp = whole.enter_context(tc.tile_pool(name="yp", bufs=1))
            wpp = whole.enter_context(tc.tile_pool(name="wpp", bufs=16))
            outp = whole.enter_context(tc.tile_pool(name="outp", bufs=2))

            for b in range(B):
                qrot = [
                    store.tile([128, T], BF, tag=f"qrot{hl}", name=f"qrot{hl}_{b}")
                    for hl in range(HL)
                ]
                krot = [
                    store.tile([128, T], BF, tag=f"krot{hl}", name=f"krot{hl}_{b}")
                    for hl in range(HL)
                ]
                v_all = store.tile(
                    [128, HL, NKC, HD], BF, tag="v_all", name=f"v_all_{b}"
                )

                # ---- phase 1: qkv projection + rope (per tb supertile) ----
                for tb in range(NTB):
                    toff = b * T + tb * TS
                    xt_t = xtp.tile([128, NE, TS], BF, tag="xt", name=f"xt_{b}_{tb}")
                    nc.sync.dma_start(
                        out=xt_t[:],
                        in_=xT[:, toff : toff + TS].rearrange("(c p) t -> p c t", p=128),
                    )
                    pqk = [
                        psum.tile([128, TS], FP, tag=f"bank{g}", name=f"qk{g}_{b}_{tb}")
                        for g in range(4)
                    ]
                    pv = [
                        psum.tile(
                            [128, 2 * HD], FP, tag=f"bank{4 + st}", name=f"v{st}_{b}_{tb}"
                        )
                        for st in range(4)
                    ]
                    for e in range(NE):
                        for g in range(4):
                            nc.tensor.matmul(
                                pqk[g][:],
                                lhsT=w_sb[:, e, g * 128 : (g + 1) * 128],
                                rhs=xt_t[:, e, :],
                                start=(e == 0),
                                stop=(e == NE - 1),
                                skip_group_check=True,
                            )
                        for st in range(4):
                            nc.tensor.matmul(
                                pv[st][:],
                                lhsT=xt_t[:, e, st * 128 : (st + 1) * 128],
                                rhs=w_sb[:, e, 512:768],
                                start=(e == 0),
                                stop=(e == NE - 1),
                                skip_group_check=True,
                            )
                    # rope: (even, odd) psum pairs -> rotated, assembled per head
                    csl = cs_sb[:, tb * TS : (tb + 1) * TS]
                    snl = sn_sb[:, tb * TS : (tb + 1) * TS]
                    for dst, pe_, po_ in ((qrot, pqk[0], pqk[1]), (krot, pqk[2], pqk[3])):
                        a_ = work.tile([128, TS], FP, tag="w0", name=f"a_{b}_{tb}")
                        b_ = work.tile([128, TS], FP, tag="w1", name=f"b_{b}_{tb}")
                        c_ = work.tile([128, TS], FP, tag="w2", name=f"c_{b}_{tb}")
                        d_ = work.tile([128, TS], FP, tag="w3", name=f"d_{b}_{tb}")
                        nc.vector.tensor_tensor(a_[:], pe_[:], csl, mybir.AluOpType.mult)
                        nc.vector.tensor_tensor(b_[:], po_[:], snl, mybir.AluOpType.mult)
                        nc.vector.tensor_tensor(c_[:], pe_[:], snl, mybir.AluOpType.mult)
                        nc.vector.tensor_tensor(d_[:], po_[:], csl, mybir.AluOpType.mult)
                        tsl = slice(tb * TS, (tb + 1) * TS)
                        for hl in range(HL):
                            hs = slice(hl * 64, (hl + 1) * 64)
                            nc.vector.tensor_tensor(
                                dst[hl][0:64, tsl], a_[hs, :], b_[hs, :],
                                mybir.AluOpType.subtract,
                            )
                            nc.vector.tensor_tensor(
                                dst[hl][64:128, tsl], c_[hs, :], d_[hs, :],
                                mybir.AluOpType.add,
                            )
                    for st in range(4):
                        j = tb * 4 + st
                        for hl in range(HL):
                            nc.vector.tensor_copy(
                                out=v_all[:, hl, j, :],
                                in_=pv[st][:, hl * HD : (hl + 1) * HD],
                            )

                # ---- phase 2: attention for this batch ----
                for tb in range(NTB):
                    for hl in range(HL):
                        po_ = psum.tile([128, TS], FP, tag="bank2", name=f"o_{b}_{hl}_{tb}")
                        psum_s = psum.tile([1, TS], FP, tag="bank3", name=f"sm_{b}_{hl}_{tb}")
                        nj = 4 * tb + 4
                        for j in range(nj):
                            sp = psum.tile(
                                [128, TS], FP, tag=f"bank{j % 2}", name=f"s_{b}_{hl}_{tb}_{j}"
                            )
                            nc.tensor.matmul(
                                sp[:],
                                lhsT=krot[hl][:, j * 128 : (j + 1) * 128],
                                rhs=qrot[hl][:, tb * TS : (tb + 1) * TS],
                                start=True,
                                stop=True,
                                skip_group_check=True,
                            )
                            ex = exps.tile([128, TS], BF, tag="exp", name=f"e_{b}_{hl}_{tb}_{j}")
                            nc.scalar.activation(
                                out=ex[:], in_=sp[:],
                                func=mybir.ActivationFunctionType.Exp, scale=SCALE,
                            )
                            if j >= 4 * tb:
                                nc.gpsimd.affine_select(
                                    out=ex[:], in_=ex[:],
                                    pattern=[[1, TS]],
                                    compare_op=mybir.AluOpType.is_ge,
                                    fill=0.0,
                                    base=tb * TS - j * 128,
                                    channel_multiplier=-1,
                                )
                            nc.tensor.matmul(
                                po_[:],
                                lhsT=v_all[:, hl, j, :],
                                rhs=ex[:],
                                start=(j == 0),
                                stop=(j == nj - 1),
                                skip_group_check=True,
                            )
                            nc.tensor.matmul(
                                psum_s[:],
                                lhsT=ones[:],
                                rhs=ex[:],
                                start=(j == 0),
                                stop=(j == nj - 1),
                                skip_group_check=True,
                            )
                        rec = recp.tile([1, TS], FP, tag="rec", name=f"r_{b}_{hl}_{tb}")
                        nc.vector.reciprocal_approx_fast(out=rec[:], in_=psum_s[:])
                        bc = bcp.tile([128, TS], FP, tag="bc", name=f"bc_{b}_{hl}_{tb}")
                        nc.gpsimd.partition_broadcast(bc[:], rec[:])
                        osb = osbp.tile([128, TS], BF, tag="osb", name=f"ot_{b}_{hl}_{tb}")
                        nc.vector.tensor_tensor(osb[:], po_[:], bc[:], mybir.AluOpType.mult)
                        nc.sync.dma_start(
                            out=a2a_in[b * NTB + tb, hl * HD : (hl + 1) * HD, :],
                            in_=osb[:],
                        )

            # ---- all-to-all: heads -> sequence re-shard ----
            nc.gpsimd.collective_compute(
                "AllToAll",
                mybir.AluOpType.bypass,
                replica_groups=[list(range(NCORES))],
                ins=[a2a_in[:].opt()],
                outs=[a2a_out[:].opt()],
            )

            # ---- phase 3: output projection for the local 512-token slice ----
            y = []
            for ci in range(NE):
                yt = yp.tile([128, TS], BF, tag=f"y{ci}", name=f"y{ci}")
                nc.sync.dma_start(
                    out=yt[:],
                    in_=a2a_out[ci // HL, (ci % HL) * HD : (ci % HL + 1) * HD, :],
                )
                y.append(yt)
            for dq in range(4):
                pp = [
                    psum.tile([128, TS], FP, tag=f"bank{t2}", name=f"pp{t2}_{dq}")
                    for t2 in range(4)
                ]
                for ci in range(NE):
                    wpt = wpp.tile([128, TS], BF, tag="wp", name=f"wp_{dq}_{ci}")
                    nc.sync.dma_start(
                        out=wpt[:],
                        in_=wpT[ci * 128 : (ci + 1) * 128, dq * TS : (dq + 1) * TS],
                    )
                    for t2 in range(4):
                        nc.tensor.matmul(
                            pp[t2][:],
                            lhsT=y[ci][:, t2 * 128 : (t2 + 1) * 128],
                            rhs=wpt[:],
                            start=(ci == 0),
                            stop=(ci == NE - 1),
                            skip_group_check=True,
                        )
                for t2 in range(4):
                    ob = outp.tile([128, TS], FP, tag="ob", name=f"ob_{dq}_{t2}")
                    nc.vector.tensor_copy(out=ob[:], in_=pp[t2][:])
                    nc.sync.dma_start(
                        out=out_loc[t2 * 128 : (t2 + 1) * 128, dq * TS : (dq + 1) * TS],
                        in_=ob[:],
                    )

    nc.compile()
    _built["nc"] = nc
    return nc


def _host_prep(x, w_attn, w_proj):
    bf = ml_dtypes.bfloat16
    x2 = np.ascontiguousarray(x.reshape(BT, D).T.astype(bf))  # [D, BT] e-major
    wpT_full = np.ascontiguousarray(w_proj.T.astype(bf))      # [c, d]

    inv = 1.0 / (10000.0 ** (np.arange(0, HD, 2, dtype=np.float32) / HD))
    t = np.arange(T, dtype=np.float32)
    fr = np.outer(t, inv)                          # [T, 64]
    cosT = np.cos(fr).T.astype(np.float32)         # [64, T]
    sinT = np.sin(fr).T.astype(np.float32)
    cs2v = np.ascontiguousarray(np.vstack([cosT, cosT]))
    sn2v = np.ascontiguousarray(np.vstack([sinT, sinT]))

    perm = np.concatenate([np.arange(0, HD, 2), np.arange(1, HD, 2)])
    in_maps = []
    for r in range(NCORES):
        h0, h1 = HL * r, HL * r + 1
        rows = []
        for off in (0, D):  # q block then k block
            rows += [off + h0 * HD + perm[:64], off + h1 * HD + perm[:64]]
            rows += [off + h0 * HD + perm[64:], off + h1 * HD + perm[64:]]
        rows += [2 * D + h0 * HD + np.arange(HD), 2 * D + h1 * HD + np.arange(HD)]
        w_c = w_attn[np.concatenate(rows)]         # [768, D]
        wT_c = np.ascontiguousarray(w_c.T.astype(bf))  # [D, 768]
        in_maps.append(
            {"xT": x2, "wT": wT_c, "wpT": wpT_full, "cs2": cs2v, "sn2": sn2v}
        )
    return in_maps


def kernel(x, w_attn, w_proj):
    global LAST_EXEC_NS, LAST_TRACE
    x = np.asarray(x, dtype=np.float32)
    w_attn = np.asarray(w_attn, dtype=np.float32)
    w_proj = np.asarray(w_proj, dtype=np.float32)

    trace = os.environ.get("KERNEL_TRACE") == "1"
    if trace:
        _install_ntff_shim()

    nc = _build()
    in_maps = _host_prep(x, w_attn, w_proj)
    kw = {}
    if trace:
        tmpdir = os.environ.get("KERNEL_TRACE_DIR") or tempfile.mkdtemp(prefix="ktrace_")
        kw = dict(trace=True, tmpdir=tmpdir)
        LAST_TRACE = tmpdir
    res = run_bass_kernel_spmd(nc, in_maps, list(range(NCORES)), **kw)
    LAST_EXEC_NS = res.exec_time_ns

    out = np.empty((B, T, D), dtype=np.float32)
    for r in range(NCORES):
        b, tb = divmod(r, NTB)
        out[b, tb * TS : (tb + 1) * TS, :] = res.results[r]["out_loc"]
    return out


# revision 6
# speedup vs baseline: 1.5459x; 1.0971x over previous
"""Causal self-attention with RoPE on 8 trn2 NeuronCores.

Sharding: heads 2r,2r+1 -> core r (both batches). w_attn column-sharded
(rows permuted even/odd per head so interleaved RoPE becomes rotate-half);
attention computed per-core in transposed [tk, tq] score layout; AllToAll
re-shards heads->sequence so each core runs the full output projection for
its own 512-token slice. Host only slices/permutes/casts inputs and
concatenates the 8 output slices. Matmul operands are bf16 (fp32 PSUM
accumulation); fp32 everywhere else.
"""

import math
import os
import sys
import tempfile

if "/opt/trn_rl_repo" not in sys.path:
    sys.path.insert(0, "/opt/trn_rl_repo")

import ml_dtypes
import numpy as np

import concourse.bacc as bacc
import concourse.bass as bass
import concourse.mybir as mybir
import concourse.tile as tile
from concourse.bass_utils import run_bass_kernel_spmd

B, T, D = 2, 2048, 2048
H, HD = 16, 128
NCORES = 8
HL = H // NCORES          # heads per core
BT = B * T
TS = 512                  # t supertile (psum bank width in fp32)
NTB = T // TS             # supertiles per batch
NE = D // 128             # e-chunks (contraction) in qkv
NKC = T // 128            # tk chunks per batch
FQKV = 3 * HL * HD        # 768 qkv features per core
FP = mybir.dt.float32
BF = mybir.dt.bfloat16
SCALE = 1.0 / math.sqrt(HD)

LAST_EXEC_NS = None
LAST_TRACE = None

_built = {}


def _install_ntff_shim():
    import types

    import antenv

    if "antenv.axon_hooks" not in sys.modules:
        mod = types.ModuleType("antenv.axon_hooks")
        _hook = [None]
        mod.set_axon_ntff_profile_hook = lambda h: _hook.__setitem__(0, h)
        mod.get_axon_ntff_profile_hook = lambda: _hook[0]
        sys.modules["antenv.axon_hooks"] = mod
        antenv.axon_hooks = mod
    from antenv.axon_hooks import (
        get_axon_ntff_profile_hook,
        set_axon_ntff_profile_hook,
    )

    if get_axon_ntff_profile_hook() is None:
        from trn_agent_boot.trn_boot import _ntff_profile_via_ctypes

        set_axon_ntff_profile_hook(_ntff_profile_via_ctypes("/opt/axon/libaxon_pjrt.so"))
    import concourse.bass_utils as bu

    bu.upload_artifacts = lambda tmpdir: f"local:{tmpdir}"


def _build():
    if "nc" in _built:
        return _built["nc"]
    nc = bacc.Bacc("TRN2", target_bir_lowering=False, debug=False, num_devices=NCORES)

    xT = nc.dram_tensor("xT", [D, BT], BF, kind="ExternalInput")
    wT = nc.dram_tensor("wT", [D, FQKV], BF, kind="ExternalInput")
    wpT = nc.dram_tensor("wpT", [D, D], BF, kind="ExternalInput")
    cs2 = nc.dram_tensor("cs2", [128, T], FP, kind="ExternalInput")
    sn2 = nc.dram_tensor("sn2", [128, T], FP, kind="ExternalInput")
    out_loc = nc.dram_tensor("out_loc", [TS, D], FP, kind="ExternalOutput")

    from contextlib import ExitStack

    with tile.TileContext(nc) as tc:
        with ExitStack() as whole:
            dpool = whole.enter_context(tc.tile_pool(name="dram", bufs=1, space="DRAM"))
            a2a_in = [
                dpool.tile([NCORES, HD, TS], BF, name=f"a2a_in{hl}")
                for hl in range(HL)
            ]
            a2a_out = [
                dpool.tile([NCORES, HD, TS], BF, name=f"a2a_out{hl}")
                for hl in range(HL)
            ]
            psum = whole.enter_context(tc.tile_pool(name="psum", bufs=1, space="PSUM"))
            smallp = whole.enter_context(tc.tile_pool(name="small", bufs=1))
            ones_f = smallp.tile([128, 1], FP, name="ones_f")
            nc.vector.memset(ones_f[:], 1.0)
            ones = smallp.tile([128, 1], BF, name="ones")
            nc.vector.tensor_copy(out=ones[:], in_=ones_f[:])

            wq = whole.enter_context(tc.tile_pool(name="wq", bufs=1))
            w_sb = wq.tile([128, NE, FQKV], BF, name="w_sb")
            for c4 in range(4):
                nc.sync.dma_start(
                    out=w_sb[:, c4 * 4 : (c4 + 1) * 4, :],
                    in_=wT[c4 * 512 : (c4 + 1) * 512, :].rearrange(
                        "(c p) f -> p c f", p=128
                    ),
                )

            tabs = whole.enter_context(tc.tile_pool(name="tabs", bufs=1))
            cs_sb = tabs.tile([128, T], FP, name="cs_sb")
            sn_sb = tabs.tile([128, T], FP, name="sn_sb")
            nc.sync.dma_start(out=cs_sb[:], in_=cs2[:])
            nc.sync.dma_start(out=sn_sb[:], in_=sn2[:])

            store = whole.enter_context(tc.tile_pool(name="store", bufs=1))
            xtp = whole.enter_context(tc.tile_pool(name="xt", bufs=2))
            work = whole.enter_context(tc.tile_pool(name="work", bufs=2))
            exps = whole.enter_context(tc.tile_pool(name="exps", bufs=5))
            osbp = whole.enter_context(tc.tile_pool(name="osb", bufs=2))
            bcp = whole.enter_context(tc.tile_pool(name="bc", bufs=2))
            recp = whole.enter_context(tc.tile_pool(name="rec", bufs=2))
            yp = whole.enter_context(tc.tile_pool(name="yp", bufs=1))
            wpp = whole.enter_context(tc.tile_pool(name="wpp", bufs=16))
            outp = whole.enter_context(tc.tile_pool(name="outp", bufs=2))

            qrots, krots, v_alls = {}, {}, {}
            for b in range(B):
                qrot = [
                    store.tile([128, T], BF, tag=f"qrot{hl}_{b}", name=f"qrot{hl}_{b}")
                    for hl in range(HL)
                ]
                krot = [
                    store.tile([128, T], BF, tag=f"krot{hl}_{b}", name=f"krot{hl}_{b}")
                    for hl in range(HL)
                ]
                v_all = store.tile(
                    [128, HL, NKC, HD], BF, tag=f"v_all_{b}", name=f"v_all_{b}"
                )
                qrots[b], krots[b], v_alls[b] = qrot, krot, v_all

                # ---- phase 1: qkv projection + rope (per tb supertile) ----
                for tb in range(NTB):
                    toff = b * T + tb * TS
                    xt_t = xtp.tile([128, NE, TS], BF, tag="xt", name=f"xt_{b}_{tb}")
                    nc.gpsimd.dma_start(
                        out=xt_t[:],
                        in_=xT[:, toff : toff + TS].rearrange("(c p) t -> p c t", p=128),
                    )
                    pqk = [
                        psum.tile([128, TS], FP, tag=f"bank{g}", name=f"qk{g}_{b}_{tb}")
                        for g in range(4)
                    ]
                    pv = [
                        psum.tile(
                            [128, 2 * HD], FP, tag=f"bank{4 + st}", name=f"v{st}_{b}_{tb}"
                        )
                        for st in range(4)
                    ]
                    for e in range(NE):
                        for g in range(4):
                            nc.tensor.matmul(
                                pqk[g][:],
                                lhsT=w_sb[:, e, g * 128 : (g + 1) * 128],
                                rhs=xt_t[:, e, :],
                                start=(e == 0),
                                stop=(e == NE - 1),
                                skip_group_check=True,
                            )
                        for st in range(4):
                            nc.tensor.matmul(
                                pv[st][:],
                                lhsT=xt_t[:, e, st * 128 : (st + 1) * 128],
                                rhs=w_sb[:, e, 512:768],
                                start=(e == 0),
                                stop=(e == NE - 1),
                                skip_group_check=True,
                            )
                    # rope: (even, odd) psum pairs -> rotated, assembled per head
                    csl = cs_sb[:, tb * TS : (tb + 1) * TS]
                    snl = sn_sb[:, tb * TS : (tb + 1) * TS]
                    for dst, pe_, po_ in ((qrot, pqk[0], pqk[1]), (krot, pqk[2], pqk[3])):
                        a_ = work.tile([128, TS], FP, tag="w0", name=f"a_{b}_{tb}")
                        b_ = work.tile([128, TS], FP, tag="w1", name=f"b_{b}_{tb}")
                        c_ = work.tile([128, TS], FP, tag="w2", name=f"c_{b}_{tb}")
                        d_ = work.tile([128, TS], FP, tag="w3", name=f"d_{b}_{tb}")
                        nc.vector.tensor_tensor(a_[:], pe_[:], csl, mybir.AluOpType.mult)
                        nc.vector.tensor_tensor(b_[:], po_[:], snl, mybir.AluOpType.mult)
                        nc.vector.tensor_tensor(c_[:], pe_[:], snl, mybir.AluOpType.mult)
                        nc.vector.tensor_tensor(d_[:], po_[:], csl, mybir.AluOpType.mult)
                        tsl = slice(tb * TS, (tb + 1) * TS)
                        for hl in range(HL):
                            hs = slice(hl * 64, (hl + 1) * 64)
                            nc.vector.tensor_tensor(
                                dst[hl][0:64, tsl], a_[hs, :], b_[hs, :],
                                mybir.AluOpType.subtract,
                            )
                            nc.vector.tensor_tensor(
                                dst[hl][64:128, tsl], c_[hs, :], d_[hs, :],
                                mybir.AluOpType.add,
                            )
                    for st in range(4):
                        j = tb * 4 + st
                        for hl in range(HL):
                            nc.vector.tensor_copy(
                                out=v_all[:, hl, j, :],
                                in_=pv[st][:, hl * HD : (hl + 1) * HD],
                            )

            # ---- phase 2: attention, hl-outer; one all-to-all per hl ----
            y = [None] * NE
            SB = [0, 1, 4, 5]  # 4-deep score-psum rotation
            for hl in range(HL):
                for b in range(B):
                    qrot, krot, v_all = qrots[b], krots[b], v_alls[b]
                    for tb in range(NTB):
                        par = (b * NTB + tb) % 2
                        po_ = psum.tile(
                            [128, TS], FP, tag=f"bank{2 if par == 0 else 6}",
                            name=f"o_{b}_{hl}_{tb}",
                        )
                        psum_s = psum.tile(
                            [1, TS], FP, tag=f"bank{3 if par == 0 else 7}",
                            name=f"sm_{b}_{hl}_{tb}",
                        )
                        nj = 4 * tb + 4
                        for j in range(nj):
                            sp = psum.tile(
                                [128, TS], FP, tag=f"bank{SB[j % 4]}",
                                name=f"s_{b}_{hl}_{tb}_{j}",
                            )
                            nc.tensor.matmul(
                                sp[:],
                                lhsT=krot[hl][:, j * 128 : (j + 1) * 128],
                                rhs=qrot[hl][:, tb * TS : (tb + 1) * TS],
                                start=True,
                                stop=True,
                                skip_group_check=True,
                            )
                            ex = exps.tile([128, TS], BF, tag="exp", name=f"e_{b}_{hl}_{tb}_{j}")
                            nc.scalar.activation(
                                out=ex[:], in_=sp[:],
                                func=mybir.ActivationFunctionType.Exp, scale=SCALE,
                            )
                            if j >= 4 * tb:
                                nc.gpsimd.affine_select(
                                    out=ex[:], in_=ex[:],
                                    pattern=[[1, TS]],
                                    compare_op=mybir.AluOpType.is_ge,
                                    fill=0.0,
                                    base=tb * TS - j * 128,
                                    channel_multiplier=-1,
                                )
                            nc.tensor.matmul(
                                po_[:],
                                lhsT=v_all[:, hl, j, :],
                                rhs=ex[:],
                                start=(j == 0),
                                stop=(j == nj - 1),
                                skip_group_check=True,
                            )
                            nc.tensor.matmul(
                                psum_s[:],
                                lhsT=ones[:],
                                rhs=ex[:],
                                start=(j == 0),
                                stop=(j == nj - 1),
                                skip_group_check=True,
                            )
                        rec = recp.tile([1, TS], FP, tag="rec", name=f"r_{b}_{hl}_{tb}")
                        nc.vector.reciprocal_approx_fast(out=rec[:], in_=psum_s[:])
                        bc = bcp.tile([128, TS], FP, tag="bc", name=f"bc_{b}_{hl}_{tb}")
                        nc.gpsimd.partition_broadcast(bc[:], rec[:])
                        osb = osbp.tile([128, TS], BF, tag="osb", name=f"ot_{b}_{hl}_{tb}")
                        nc.vector.tensor_tensor(osb[:], po_[:], bc[:], mybir.AluOpType.mult)
                        nc.sync.dma_start(
                            out=a2a_in[hl][b * NTB + tb, :, :],
                            in_=osb[:],
                        )
                # heads->sequence re-shard for this hl (overlaps next hl's attention)
                nc.gpsimd.collective_compute(
                    "AllToAll",
                    mybir.AluOpType.bypass,
                    replica_groups=[list(range(NCORES))],
                    ins=[a2a_in[hl][:].opt()],
                    outs=[a2a_out[hl][:].opt()],
                )
                for src in range(NCORES):
                    ci = HL * src + hl
                    yt = yp.tile([128, TS], BF, tag=f"y{ci}", name=f"y{ci}")
                    nc.sync.dma_start(out=yt[:], in_=a2a_out[hl][src, :, :])
                    y[ci] = yt

            # ---- phase 3: output projection for the local 512-token slice ----
            for dq in range(4):
                pp = [
                    psum.tile([128, TS], FP, tag=f"bank{t2}", name=f"pp{t2}_{dq}")
                    for t2 in range(4)
                ]
                for ci in range(NE):
                    wpt = wpp.tile([128, TS], BF, tag="wp", name=f"wp_{dq}_{ci}")
                    nc.sync.dma_start(
                        out=wpt[:],
                        in_=wpT[ci * 128 : (ci + 1) * 128, dq * TS : (dq + 1) * TS],
                    )
                    for t2 in range(4):
                        nc.tensor.matmul(
                            pp[t2][:],
                            lhsT=y[ci][:, t2 * 128 : (t2 + 1) * 128],
                            rhs=wpt[:],
                            start=(ci == 0),
                            stop=(ci == NE - 1),
                            skip_group_check=True,
                        )
                for t2 in range(4):
                    ob = outp.tile([128, TS], FP, tag="ob", name=f"ob_{dq}_{t2}")
                    nc.vector.tensor_copy(out=ob[:], in_=pp[t2][:])
                    nc.sync.dma_start(
                        out=out_loc[t2 * 128 : (t2 + 1) * 128, dq * TS : (dq + 1) * TS],
                        in_=ob[:],
                    )

    nc.compile()
    _built["nc"] = nc
    return nc


def _host_prep(x, w_attn, w_proj):
    bf = ml_dtypes.bfloat16
    x2 = np.ascontiguousarray(x.reshape(BT, D).T.astype(bf))  # [D, BT] e-major
    wpT_full = np.ascontiguousarray(w_proj.T.astype(bf))      # [c, d]

    inv = 1.0 / (10000.0 ** (np.arange(0, HD, 2, dtype=np.float32) / HD))
    t = np.arange(T, dtype=np.float32)
    fr = np.outer(t, inv)                          # [T, 64]
    cosT = np.cos(fr).T.astype(np.float32)         # [64, T]
    sinT = np.sin(fr).T.astype(np.float32)
    cs2v = np.ascontiguousarray(np.vstack([cosT, cosT]))
    sn2v = np.ascontiguousarray(np.vstack([sinT, sinT]))

    perm = np.concatenate([np.arange(0, HD, 2), np.arange(1, HD, 2)])
    in_maps = []
    for r in range(NCORES):
        h0, h1 = HL * r, HL * r + 1
        rows = []
        for off in (0, D):  # q block then k block
            rows += [off + h0 * HD + perm[:64], off + h1 * HD + perm[:64]]
            rows += [off + h0 * HD + perm[64:], off + h1 * HD + perm[64:]]
        rows += [2 * D + h0 * HD + np.arange(HD), 2 * D + h1 * HD + np.arange(HD)]
        w_c = w_attn[np.concatenate(rows)]         # [768, D]
        wT_c = np.ascontiguousarray(w_c.T.astype(bf))  # [D, 768]
        in_maps.append(
            {"xT": x2, "wT": wT_c, "wpT": wpT_full, "cs2": cs2v, "sn2": sn2v}
        )
    return in_maps


def kernel(x, w_attn, w_proj):
    global LAST_EXEC_NS, LAST_TRACE
    x = np.asarray(x, dtype=np.float32)
    w_attn = np.asarray(w_attn, dtype=np.float32)
    w_proj = np.asarray(w_proj, dtype=np.float32)

    trace = os.environ.get("KERNEL_TRACE") == "1"
    if trace:
        _install_ntff_shim()

    nc = _build()
    in_maps = _host_prep(x, w_attn, w_proj)
    kw = {}
    if trace:
        tmpdir = os.environ.get("KERNEL_TRACE_DIR") or tempfile.mkdtemp(prefix="ktrace_")
        kw = dict(trace=True, tmpdir=tmpdir)
        LAST_TRACE = tmpdir
    res = run_bass_kernel_spmd(nc, in_maps, list(range(NCORES)), **kw)
    LAST_EXEC_NS = res.exec_time_ns

    out = np.empty((B, T, D), dtype=np.float32)
    for r in range(NCORES):
        b, tb = divmod(r, NTB)
        out[b, tb * TS : (tb + 1) * TS, :] = res.results[r]["out_loc"]
    return out
